# revision 32
# baseline (speedup 1.0000x reference)
"""DBSS block as ONE fused SPMD bass launch on 8 NeuronCores.

Core i handles batch b=i//4 and either row-quarter q=i%4 (phases A/C) or
scan direction k=i%4 (phase B). Resharding between phases happens on
device via batch-local AllGathers; the snake permutations are done on
device with parity/reversal/transpose copies selected by per-core mask
scalars (so the SPMD program is identical on every core).
"""
import os, sys
for _p in ('/opt/trn_rl_repo', os.path.expanduser('~/.axon_site/_ro/trn_rl_repo')):
    if os.path.isdir(_p) and _p not in sys.path:
        sys.path.insert(0, _p)

import numpy as np
from contextlib import ExitStack
import concourse.bass as bass
import concourse.mybir as mybir
from concourse import tile

# ------------------------------------------------------------------ walrus
# The walrus build in this container rejects TPB_CTRL instructions carrying
# more than one semaphore wait. Tile's kernel-tail drain aggregates one wait
# per live semaphore; split that drain into single-wait carriers. The
# carriers write into the pre-pool mw_scratch tensor (NOT a freshly
# allocated one: at drain time the pool has restored sbuf_base, so a fresh
# alloc would land inside still-live pool memory and corrupt it).
_orig_drain_and_barrier = tile.TileContext._drain_and_barrier

def _split_drain_and_barrier(self, tick_clock, wait_clock):
    from concourse.vector_clock import ScopedClock
    import bass_rust as _br
    probe = self.nc.sync.drain()
    wait_clock.add_sem_waits(probe.ins, ScopedClock({None: tick_clock.global_clock}))
    waits = list(probe.ins.sync_info.on_wait) if probe.ins.sync_info else []
    if waits:
        probe.ins.sync_info = _br.SyncInfo(on_wait=[], on_update=[])
        scratch = self.nc._mw_scratch
        for wi, w in enumerate(waits):
            col = 448 + (wi % 64)
            ins = self.nc.vector.memset(scratch.ap()[:, col:col + 1], 0.0)
            ins.ins.sync_info = _br.SyncInfo(on_wait=[w], on_update=[])
    self.nc.all_engine_barrier()
    assert self.sems is not None
    popped = self.nc._tile_sem_poison_stack.pop()
    assert popped is self._sem_poison
    self.nc.clear_and_free_semaphores(list(self.sems.allocated().values()))
    self.nc.all_engine_barrier()

tile.TileContext._drain_and_barrier = _split_drain_and_barrier


def _new_bass():
    nc = bass.Bass()
    nc._mw_scratch = nc.alloc_sbuf_tensor("mw_scratch", [1, 512], mybir.dt.float32)
    nc._mw_sems = [nc.alloc_semaphore(f"mw_sem_{i}") for i in range(64)]
    return nc


def _fix_multiwaits(nc):
    """Distribute extra sem waits over single-wait DVE memset carriers.

    Helper semaphores are drawn round-robin from a fixed pool and reused
    with cumulative thresholds (use #n of a sem waits for sem >= n), so the
    pool never exhausts."""
    import bass_rust as _br
    scratch = nc._mw_scratch
    helper_sems = []
    sem_uses = {}
    sem_rr = [0]
    scri = [0]
    for bbw in nc.main_func.blocks:
        insns = bbw.instructions
        out = []
        for ins in insns:
            si = ins.sync_info
            waits = list(si.on_wait) if si else []
            if len(waits) <= 1:
                out.append(ins)
                continue
            eng = str(ins.engine)
            mk = []
            def carrier(w, upd=None):
                si_ = scri[0] % 448
                scri[0] += 1
                c = mybir.InstMemset(name=nc.get_next_instruction_name(),
                                     mode="Const", constant=0, ins=[],
                                     outs=[nc.vector.lower_ap(scratch.ap()[:, si_:si_ + 1])])
                c.engine = ins.engine if eng in ("EngineType.DVE", "EngineType.Pool") else mybir.EngineType.DVE
                c.sync_info = _br.SyncInfo(on_wait=[w] if w else [],
                                           on_update=[upd] if upd else [])
                nc.register_instruction(c, overwrite=True)
                mk.append(c)
            if eng in ("EngineType.DVE", "EngineType.Pool"):
                for w in waits[:-1]:
                    carrier(w)
                ins.sync_info = _br.SyncInfo(on_wait=[waits[-1]],
                                             on_update=list(si.on_update) if si else [])
            else:
                sem = nc._mw_sems[sem_rr[0] % len(nc._mw_sems)]
                sem_rr[0] += 1
                cnt = sem_uses.get(sem.num, 0) + 1
                sem_uses[sem.num] = cnt
                if cnt == 1:
                    helper_sems.append(sem)
                for w in waits[:-1]:
                    carrier(w)
                carrier(waits[-1],
                        _br.SyncUpdate(sync_type='semaphore', id=sem.num,
                                       ant_name=sem.name, update_mode='sem-inc',
                                       update_value=1, update_reg=None))
                ins.sync_info = _br.SyncInfo(
                    on_wait=[_br.SyncWait(sync_type='semaphore', id=sem.num,
                                          ant_name=sem.name, wait_mode='sem-ge-imm',
                                          wait_value=cnt, wait_reg=None)],
                    on_update=list(si.on_update) if si else [])
            out.extend(mk)
            out.append(ins)
        bbw.instructions = out
    if helper_sems:
        from concourse.bass import compact_to_ranges as _ctr
        nums = [s.num for s in helper_sems]
        first_bb = nc.main_func.blocks[0]
        home = nc.cur_bb.bb
        n0 = len(home.instructions)
        try:
            rngs = _ctr(sorted(nums))
        except Exception:
            rngs = [range(n, n + 1) for n in sorted(nums)]
        for r in rngs:
            nc.gpsimd.sem_clear(r)
        lst = home.instructions
        head_clears = lst[n0:]
        home.instructions = lst[:n0]
        first_bb.instructions = head_clears + first_bb.instructions
        for r in rngs:
            nc.gpsimd.sem_clear(r)


F32 = mybir.dt.float32
F32R = mybir.dt.float32r
BF16 = mybir.dt.bfloat16
AL = mybir.AluOpType
AF = mybir.ActivationFunctionType

B, C, H, W = 2, 64, 64, 64
D2, L, N, R = 32, H * W, 16, 2
HID = 256
EPS = 1e-5
NCORE = 8
NPIX = float(B * H * W)
NPB = float(H * W)
DBG = False
XS_BF16 = True    # ship the x slices in bf16 (halves the per-call upload)

# ============================================================ device program

def build_fused(dbg=False):
    nc = _new_bass()
    inp = {}
    def I(nm, shp, dt=F32):
        inp[nm] = nc.dram_tensor(nm, shp, dt, kind="ExternalInput")
        return inp[nm]

    # phase A. xall carries ALL cores' x slices; only core 0's copy holds
    # real data (the rest are device-resident zeros) — a ReduceScatter(add)
    # hands slice i to core i, so the host does a single-shard upload.
    I("xall", [NCORE * C, 22, W], BF16 if XS_BF16 else F32)
    XDT = BF16 if XS_BF16 else F32
    xall_i = nc.dram_tensor("xall_i", [NCORE * C, 22, W], XDT)
    xsl = nc.dram_tensor("xsl", [C, 22, W], XDT)
    I("l1mask", [128, 12]); I("selsum", [128, 2]); I("selg", [2, 128])
    I("b2", [128, 1]); I("w9", [128, 9]); I("cb", [128, 1]); I("cst2", [2, 2])
    # phase B (per-direction k)
    I("WdT", [D2, D2]); I("bgen", [D2, 16]); I("cgen", [D2, 16])
    I("bias4", [128, 1]); I("A4", [128, 4]); I("dsc", [D2, 1])
    I("i32", [D2, D2]); I("kmask", [D2, 4])
    # phase C
    I("rowmask14", [128, 14]); I("bmask", [128, 8]); I("qmask", [128, 4])
    I("selhp", [128, 128]); I("ecaT", [128, 128])
    I("sg_ssm", [2, 128]); I("sb_ssm", [128, 1])
    I("sg_ln2", [2, 128]); I("sb_ln2", [128, 1])
    I("projc", [64, 64]); I("projb", [128, 1])
    I("bnp_g", [128, 1]); I("bnp_b", [128, 1])
    I("fc1c", [64, 256]); I("bn1_g", [128, 4]); I("bn1_b", [128, 4])
    I("dwc", [64, 4, 49]); I("dwb", [128, 4])
    I("bn2_g", [128, 4]); I("bn2_b", [128, 4])
    I("fc2c", [64, 4, 64]); I("bn3_g", [128, 1]); I("bn3_b", [128, 1])
    I("cstc", [128, 2])

    outg = nc.dram_tensor("outg", [NCORE * C, 16 * W], BF16, kind="ExternalOutput")
    og_in = nc.dram_tensor("og_in", [C, 16 * W], BF16)
    og_cc = nc.dram_tensor("og_cc", [NCORE * C, 16 * W], BF16)
    hcc_in = nc.dram_tensor("hcc_in", [C, 16 * W], F32)
    hcc_out = nc.dram_tensor("hcc_out", [4 * C, 16 * W], F32)
    ycc_in = nc.dram_tensor("ycc_in", [D2, L], F32)
    ycc_out = nc.dram_tensor("ycc_out", [4 * D2, L], F32)
    scc_in = [nc.dram_tensor(f"sccin{r}", [128, 16], F32) for r in range(4)]
    scc_out = [nc.dram_tensor(f"sccout{r}", [NCORE * 128, 16], F32) for r in range(4)]
    if dbg:
        h_dbg = nc.dram_tensor("h_dbg", [C, 16 * W], F32, kind="ExternalOutput")
        u_dbg = nc.dram_tensor("u_dbg", [D2, L], F32, kind="ExternalOutput")
        y_dbg = nc.dram_tensor("y_dbg", [D2, L], F32, kind="ExternalOutput")
        ym_dbg = nc.dram_tensor("ym_dbg", [128, 14, 64], F32, kind="ExternalOutput")
        x1_dbg = nc.dram_tensor("x1_dbg", [128, 14, 70], F32, kind="ExternalOutput")

    GRP_B = [[0, 1, 2, 3], [4, 5, 6, 7]]
    GRP_ALL = [list(range(NCORE))]

    with tile.TileContext(nc, linearize=True) as tc:
        # ============================================ phase A: ln1 + dwconv
        with ExitStack() as ctx:
            pool = ctx.enter_context(tc.tile_pool(name="poolA", bufs=1))
            psum = ctx.enter_context(tc.tile_pool(name="psumA", bufs=1, space="PSUM"))
            # bounce the IO tensor into an internal one (collectives cannot
            # read IO tensors), then scatter slice i to core i
            xbnc = pool.tile([128, 4, 22 * W], XDT)
            for blk in range(4):
                nc.sync.dma_start(xbnc[:, blk, :],
                                  inp["xall"][128 * blk:128 * blk + 128]
                                  .rearrange("p a b -> p (a b)"))
            for blk in range(4):
                nc.sync.dma_start(xall_i[128 * blk:128 * blk + 128]
                                  .rearrange("p a b -> p (a b)"), xbnc[:, blk, :])
            nc.gpsimd.collective_compute("ReduceScatter", AL.add, replica_groups=GRP_ALL,
                                         ins=[xall_i[:]], outs=[xsl[:]])
            T = {}
            for nm in ("l1mask", "selsum", "selg", "b2", "w9", "cb", "cst2"):
                T[nm] = pool.tile(list(inp[nm].shape), F32, name=f"tA_{nm}")
                nc.sync.dma_start(T[nm][:], inp[nm][:])
            xt = pool.tile([128, 12, 68], F32)
            nc.vector.memset(xt[:], 0.0)
            if XS_BF16:
                xbf = pool.tile([128, 12, 64], BF16)
                for h in (0, 1):
                    nc.sync.dma_start(xbf[64 * h:64 * h + 64, :, :],
                                      xsl[:, 1 + 8 * h:1 + 8 * h + 12, :])
                nc.vector.tensor_copy(xt[:, :, 2:66], xbf[:])
            else:
                for h in (0, 1):
                    nc.sync.dma_start(xt[64 * h:64 * h + 64, :, 2:66],
                                      xsl[:, 1 + 8 * h:1 + 8 * h + 12, :])
            XW = xt[:, :, 2:66]
            sq = pool.tile([128, 12, 64], F32)
            nc.scalar.activation(sq[:], XW, AF.Square)
            st_x = psum.tile([2, 768], F32)
            st_xx = psum.tile([2, 768], F32)
            for r0, r1 in ((0, 8), (8, 12)):
                nc.tensor.matmul(st_x[:, r0 * 64:r1 * 64], T["selsum"][:], xt[:, r0:r1, 2:66])
                nc.tensor.matmul(st_xx[:, r0 * 64:r1 * 64], T["selsum"][:], sq[:, r0:r1, :])
            sm = pool.tile([2, 768], F32)
            nc.vector.tensor_scalar(sm[:], st_x[:], 1.0 / 64, None, AL.mult)
            var = pool.tile([2, 768], F32)
            nc.vector.tensor_tensor(var[:], sm[:], sm[:], AL.mult)
            nc.vector.scalar_tensor_tensor(var[:], st_xx[:], 1.0 / 64, var[:], AL.mult, AL.subtract)
            inv = pool.tile([2, 768], F32)
            nc.scalar.activation(inv[:], var[:], AF.Ln, bias=T["cst2"][:, 0:1])
            nc.scalar.activation(inv[:], inv[:], AF.Exp, scale=-0.5)
            minv = pool.tile([2, 768], F32)
            nc.vector.tensor_tensor(minv[:], sm[:], inv[:], AL.mult)
            sgb = psum.tile([128, 12, 64], F32)
            msgb = psum.tile([128, 12, 64], F32)
            for r0, r1 in ((0, 8), (8, 12)):
                nc.tensor.matmul(sgb[:, r0:r1, :], T["selg"][:], inv[:, r0 * 64:r1 * 64])
                nc.tensor.matmul(msgb[:, r0:r1, :], T["selg"][:], minv[:, r0 * 64:r1 * 64])
            xn = pool.tile([128, 12, 64], F32)
            nc.vector.tensor_tensor(xn[:], XW, sgb[:], AL.mult)
            nc.vector.scalar_tensor_tensor(xn[:], xn[:], T["b2"][:], msgb[:], AL.add, AL.subtract)
            xmp = pool.tile([128, 12, 68], F32)
            nc.vector.memset(xmp[:], 0.0)
            nc.vector.tensor_tensor(xmp[:, :, 2:66], xn[:],
                                    T["l1mask"][:].unsqueeze(2).broadcast_to([128, 12, 64]), AL.mult)
            acc0 = pool.tile([128, 8, 64], F32)
            acc1 = pool.tile([128, 8, 64], F32)
            acc = [acc0, acc1]
            taps = [(dy, dx) for dy in (-2, 0, 2) for dx in (-2, 0, 2)]
            cur = 0
            for ti, (dy, dx) in enumerate(taps):
                src = xmp[:, 2 + dy:10 + dy, 2 + dx:66 + dx]
                if ti == 0:
                    nc.vector.tensor_scalar(acc[0][:], src, T["w9"][:, 0:1], T["cb"][:], AL.mult, AL.add)
                else:
                    nc.vector.scalar_tensor_tensor(acc[1 - cur][:], src, T["w9"][:, ti:ti + 1],
                                                   acc[cur][:], AL.mult, AL.add)
                    cur = 1 - cur
            for h in (0, 1):
                nc.sync.dma_start(hcc_in[:, 8 * 64 * h:8 * 64 * h + 8 * 64],
                                  acc[cur][64 * h:64 * h + 64].rearrange("p a b -> p (a b)"))
                if dbg:
                    nc.sync.dma_start(h_dbg[:, 8 * 64 * h:8 * 64 * h + 8 * 64],
                                      acc[cur][64 * h:64 * h + 64].rearrange("p a b -> p (a b)"))
        nc.gpsimd.collective_compute("AllGather", AL.bypass, replica_groups=GRP_B,
                                     ins=[hcc_in[:]], outs=[hcc_out[:]])

        # ============================================ phase B: snake scan
        with ExitStack() as ctx:
            pool = ctx.enter_context(tc.tile_pool(name="poolB", bufs=1))
            psA = ctx.enter_context(tc.tile_pool(name="psA", bufs=3, space="PSUM"))
            psY = ctx.enter_context(tc.tile_pool(name="psY", bufs=1, space="PSUM"))
            T = {}
            for nm in ("WdT", "bgen", "cgen", "bias4", "A4", "dsc", "i32", "kmask"):
                T[nm] = pool.tile(list(inp[nm].shape), F32, name=f"tB_{nm}")
                nc.sync.dma_start(T[nm][:], inp[nm][:])
            km = T["kmask"]

            # constructed weights
            lhsT_d = pool.tile([D2, 4, D2], F32)      # [e, j, d] = Wd[d, e]
            nc.vector.tensor_copy(lhsT_d[:], T["WdT"][:].unsqueeze(1).broadcast_to([D2, 4, D2]))
            lhsT_B = pool.tile([D2, 16, D2], F32)     # [e, gj, d]
            nc.vector.tensor_copy(lhsT_B[:], T["bgen"][:].unsqueeze(2).broadcast_to([D2, 16, D2]))
            lhsT_C = pool.tile([D2, 16, D2], F32)
            nc.vector.tensor_copy(lhsT_C[:], T["cgen"][:].unsqueeze(2).broadcast_to([D2, 16, D2]))
            lhsT_y = pool.tile([128, D2], F32)        # [32j+d, d'] = I
            nc.vector.tensor_copy(lhsT_y[0:D2, :], T["i32"][:])
            for j in (1, 2, 3):
                nc.sync.dma_start(lhsT_y[D2 * j:D2 * j + D2, :], lhsT_y[0:D2, :])

            u4 = pool.tile([128, L], F32)
            BIG = dict(tag="B16", bufs=4)

            # build u in canonical order: row-branch from ch 0:32, col from 32:64
            hic = pool.tile([D2, 64, 64], F32, name="hic", **BIG)
            for qq in range(4):
                nc.sync.dma_start(hic[:, 16 * qq:16 * qq + 16, :],
                                  hcc_out[64 * qq + D2:64 * qq + 2 * D2, :]
                                  .rearrange("p (a b) -> p a b", a=16))
            hicT = pool.tile([D2, 64, 64], F32, name="hicT", **BIG)
            nc.vector.tensor_copy(hicT[:], hic[:].rearrange("p a b -> p b a"))
            u_col = pool.tile([D2, 64, 64], F32, name="u_col", **BIG)
            nc.vector.tensor_copy(u_col[:, 0::2, :], hicT[:, 0::2, :])
            nc.vector.tensor_copy(u_col[:, 1::2, :], hicT[:, 1::2, ::-1])
            hir = pool.tile([D2, 64, 64], F32, name="hir", **BIG)
            for qq in range(4):
                nc.sync.dma_start(hir[:, 16 * qq:16 * qq + 16, :],
                                  hcc_out[64 * qq:64 * qq + D2, :]
                                  .rearrange("p (a b) -> p a b", a=16))
            u_row = pool.tile([D2, 64, 64], F32, name="u_row", **BIG)
            nc.vector.tensor_copy(u_row[:, 0::2, :], hir[:, 0::2, :])
            nc.vector.tensor_copy(u_row[:, 1::2, :], hir[:, 1::2, ::-1])
            ur = u_row[:].rearrange("p a b -> p (a b)")
            uc = u_col[:].rearrange("p a b -> p (a b)")
            nc.vector.tensor_scalar(u4[0:D2, :], ur, km[:, 0:1], None, AL.mult)
            nc.vector.scalar_tensor_tensor(u4[0:D2, :], uc, km[:, 1:2], u4[0:D2, :],
                                           AL.mult, AL.add)
            ucn = pool.tile([D2, L], F32, name="ucn", **BIG)
            nc.vector.tensor_copy(ucn[:], u4[0:D2, :])
            nc.vector.tensor_scalar(u4[0:D2, :], ucn[:], km[:, 2:3], None, AL.mult)
            nc.vector.scalar_tensor_tensor(u4[0:D2, :], ucn[:, ::-1], km[:, 3:4],
                                           u4[0:D2, :], AL.mult, AL.add)
            for j in (1, 2, 3):
                nc.sync.dma_start(u4[D2 * j:D2 * j + D2, :], u4[0:D2, :])
            if dbg:
                nc.sync.dma_start(u_dbg[:], u4[0:D2, :])

            NCH = 8
            CH = L // NCH
            ut = u4[0:D2, :]

            def mm(out_ap, lh, rh, f32r=False, **kw):
                if f32r:
                    lh, rh = lh.bitcast(F32R), rh.bitcast(F32R)
                nc.tensor.matmul(out_ap, lh, rh, **kw)

            d4 = pool.tile([128, L], F32)
            for c in range(NCH):
                dp = psA.tile([128, CH], F32, name=f"dp{c}", tag="ps")
                mm(dp[:], lhsT_d[:].rearrange("p a b -> p (a b)"), ut[:, c * CH:(c + 1) * CH])
                nc.scalar.activation(d4[:, c * CH:(c + 1) * CH], dp[:], AF.Exp,
                                     bias=T["bias4"][:])
                nc.scalar.activation(d4[:, c * CH:(c + 1) * CH],
                                     d4[:, c * CH:(c + 1) * CH], AF.Ln,
                                     bias=nc.const_aps.tensor(1.0, (128, 1)))
            hs = []
            for g in range(4):
                dBu = pool.tile([128, L], F32, name=f"dBu{g}", **BIG)
                for c in range(NCH):
                    b4 = psA.tile([128, CH], F32, name=f"b4_{g}_{c}", tag="ps")
                    mm(b4[:], lhsT_B[:].rearrange("p a b -> p (a b)")[:, g * 128:(g + 1) * 128],
                       ut[:, c * CH:(c + 1) * CH])
                    nc.vector.tensor_tensor(dBu[:, c * CH:(c + 1) * CH],
                                            d4[:, c * CH:(c + 1) * CH], b4[:], AL.mult)
                    nc.vector.tensor_tensor(dBu[:, c * CH:(c + 1) * CH],
                                            dBu[:, c * CH:(c + 1) * CH], u4[:, c * CH:(c + 1) * CH],
                                            AL.mult)
                dA = pool.tile([128, L], F32, name=f"dA{g}", **BIG)
                nc.scalar.activation(dA[:], d4[:], AF.Exp, scale=T["A4"][:, g:g + 1])
                hsg = pool.tile([128, L], F32, name=f"hs{g}", tag="hs", bufs=4)
                nc.vector.tensor_tensor_scan(hsg[:], dA[:], dBu[:], 0.0, AL.mult, AL.add)
                hs.append(hsg)

            ysbf = pool.tile([D2, L], F32, name="ysbf", tag="ysbf", bufs=1)
            for half in range(2):
                yps = psY.tile([D2, L // 2], F32, name=f"yps{half}", tag="yps")
                for g in range(4):
                    fsb = pool.tile([128, L // 2], F32, name=f"f_{half}_{g}", tag="fsb", bufs=1)
                    for cc in range(NCH // 2):
                        c = half * (NCH // 2) + cc
                        c4 = psA.tile([128, CH], F32, name=f"c4_{g}_{c}", tag="ps")
                        mm(c4[:], lhsT_C[:].rearrange("p a b -> p (a b)")[:, g * 128:(g + 1) * 128],
                           ut[:, c * CH:(c + 1) * CH])
                        if g % 2 == 0:
                            nc.vector.tensor_tensor(fsb[:, cc * CH:(cc + 1) * CH],
                                                    hs[g][:, c * CH:(c + 1) * CH], c4[:], AL.mult)
                        else:
                            c4sb = pool.tile([128, CH], F32, name=f"c4sb_{g}_{c}", tag="c4sb", bufs=2)
                            nc.scalar.copy(c4sb[:], c4[:])
                            nc.vector.tensor_tensor(fsb[:, cc * CH:(cc + 1) * CH],
                                                    hs[g][:, c * CH:(c + 1) * CH], c4sb[:], AL.mult)
                    for cc in range(NCH // 2):
                        mm(yps[:, cc * CH:(cc + 1) * CH], lhsT_y[:],
                           fsb[:, cc * CH:(cc + 1) * CH],
                           start=(g == 0), stop=(g == 3), skip_group_check=True)
                # ysbf = yps + Ds*u
                nc.vector.scalar_tensor_tensor(ysbf[:, half * (L // 2):(half + 1) * (L // 2)],
                                               ut[:, half * (L // 2):(half + 1) * (L // 2)],
                                               T["dsc"][:, 0:1], yps[:], AL.mult, AL.add)
            # un-reverse (odd k) then un-snake to image row-major
            ycn = pool.tile([D2, L], F32, name="ycn", **BIG)
            nc.vector.tensor_scalar(ycn[:], ysbf[:], km[:, 2:3], None, AL.mult)
            nc.vector.scalar_tensor_tensor(ycn[:], ysbf[:, ::-1], km[:, 3:4], ycn[:],
                                           AL.mult, AL.add)
            y3 = ycn[:].rearrange("p (a b) -> p a b", a=64)
            # parity-unsnake; the same op serves both branches (row: a=row,
            # col: a=column-run), the col branch just needs a free-dim
            # transpose afterwards.
            yri = pool.tile([D2, 64, 64], F32, name="yri", **BIG)
            nc.vector.tensor_copy(yri[:, 0::2, :], y3[:, 0::2, :])
            nc.vector.tensor_copy(yri[:, 1::2, :], y3[:, 1::2, ::-1])
            yci = pool.tile([D2, 64, 64], F32, name="yci", **BIG)
            nc.vector.tensor_copy(yci[:], yri[:].rearrange("p a b -> p b a"))
            yimg = pool.tile([D2, L], F32, name="yimg", **BIG)
            nc.vector.tensor_scalar(yimg[:], yri[:].rearrange("p a b -> p (a b)"),
                                    km[:, 0:1], None, AL.mult)
            nc.vector.scalar_tensor_tensor(yimg[:], yci[:].rearrange("p a b -> p (a b)"),
                                           km[:, 1:2], yimg[:], AL.mult, AL.add)
            nc.sync.dma_start(ycc_in[:], yimg[:])
            if dbg:
                nc.sync.dma_start(y_dbg[:], yimg[:])
        nc.gpsimd.collective_compute("AllGather", AL.bypass, replica_groups=GRP_B,
                                     ins=[ycc_in[:]], outs=[ycc_out[:]])

        # ============================================ phase C: merge + mlp
        with ExitStack() as ctx:
            pool = ctx.enter_context(tc.tile_pool(name="poolC", bufs=1))
            psT = ctx.enter_context(tc.tile_pool(name="psT", bufs=2, space="PSUM"))
            psS = ctx.enter_context(tc.tile_pool(name="psS", bufs=2, space="PSUM"))
            T = {}
            for nm in ("rowmask14", "bmask", "qmask", "selhp", "ecaT",
                       "sg_ssm", "sb_ssm", "sg_ln2", "sb_ln2", "projc", "projb",
                       "bnp_g", "bnp_b", "fc1c", "bn1_g", "bn1_b", "dwc", "dwb",
                       "bn2_g", "bn2_b", "fc2c", "bn3_g", "bn3_b", "cstc", "selsum"):
                T[nm] = pool.tile(list(inp[nm].shape), F32, name=f"tC_{nm}")
                nc.sync.dma_start(T[nm][:], inp[nm][:])
            eps_ap = T["cstc"][:, 0:1]

            # construct duplicated weights
            projT = pool.tile([128, 128], F32)
            nc.vector.memset(projT[:], 0.0)
            nc.vector.tensor_copy(projT[0:64, 0:64], T["projc"][:])
            nc.sync.dma_start(projT[64:128, 64:128], projT[0:64, 0:64])
            fc1T = pool.tile([128, 4, 128], F32)
            nc.vector.memset(fc1T[:], 0.0)
            nc.vector.tensor_copy(fc1T[0:64, :, 0:64],
                                  T["fc1c"][:].rearrange("p (a b) -> p a b", a=4))
            nc.sync.dma_start(fc1T[64:128, :, 64:128], fc1T[0:64, :, 0:64])
            fc2T = pool.tile([128, 4, 128], F32)
            nc.vector.memset(fc2T[:], 0.0)
            nc.vector.tensor_copy(fc2T[0:64, :, 0:64], T["fc2c"][:])
            nc.sync.dma_start(fc2T[64:128, :, 64:128], fc2T[0:64, :, 0:64])
            dw_w = pool.tile([128, 4, 49], F32)
            nc.vector.tensor_copy(dw_w[0:64, :, :], T["dwc"][:])
            nc.sync.dma_start(dw_w[64:128, :, :], dw_w[0:64, :, :])
            cmask = pool.tile([128, 14, 70], F32)
            nc.vector.memset(cmask[:], 0.0)
            nc.vector.tensor_copy(cmask[:, :, 3:67],
                                  T["rowmask14"][:].unsqueeze(2).broadcast_to([128, 14, 64]))

            RW = 70
            OWN = (slice(3, 11), slice(3, 67))

            # ---- window variants + direction merge + q-select into ym
            ym = pool.tile([128, 14, RW], F32)
            nc.vector.memset(ym[:], 0.0)
            for v in range(4):
                ygv = pool.tile([128, 28, 64], F32, name=f"ygv{v}", tag="ygv", bufs=2)
                nc.vector.memset(ygv[:], 0.0)
                rlo = 16 * v - 3
                for h in (0, 1):
                    for br in (0, 1):
                        for d2 in (0, 1):
                            g0 = max(rlo + 8 * h, 0)
                            g1 = min(rlo + 8 * h + 14, 64)
                            s0 = g0 - (rlo + 8 * h)
                            nc.sync.dma_start(
                                ygv[64 * h + 32 * br:64 * h + 32 * br + 32,
                                    14 * d2 + s0:14 * d2 + s0 + (g1 - g0), :],
                                ycc_out[32 * (2 * br + d2):32 * (2 * br + d2) + 32,
                                        64 * g0:64 * g1]
                                .rearrange("p (a b) -> p a b", b=64))
                mrg = pool.tile([128, 14, 64], F32, name=f"mrg{v}", tag="mrg", bufs=2)
                nc.vector.tensor_tensor(mrg[:], ygv[:, 0:14, :], ygv[:, 14:28, :], AL.add)
                nc.vector.scalar_tensor_tensor(ym[:, :, 3:67], mrg[:], T["qmask"][:, v:v + 1],
                                               ym[:, :, 3:67], AL.mult, AL.add)
            if dbg:
                nc.sync.dma_start(ym_dbg[:], ym[:, :, 3:67])

            xt = pool.tile([128, 14, RW], F32)
            nc.vector.memset(xt[:], 0.0)
            if XS_BF16:
                xbf2 = pool.tile([128, 14, 64], BF16)
                for h in (0, 1):
                    nc.sync.dma_start(xbf2[64 * h:64 * h + 64, :, :],
                                      xsl[:, 8 * h:8 * h + 14, :])
                nc.vector.tensor_copy(xt[:, :, 3:67], xbf2[:])
            else:
                for h in (0, 1):
                    nc.sync.dma_start(xt[64 * h:64 * h + 64, :, 3:67],
                                      xsl[:, 8 * h:8 * h + 14, :])

            def chunks2(tile3):
                return [tile3[:, 0:7, :], tile3[:, 7:14, :]]

            def ln_ch(src, selg_key, b_key, nm):
                sq = pool.tile([128, 14, RW], F32, name=f"sq_{nm}", tag="lnsq")
                nc.scalar.activation(sq[:], src[:], AF.Square)
                inv = pool.tile([2, 14, RW], F32, name=f"inv_{nm}", tag="lninv")
                minv = pool.tile([2, 14, RW], F32, name=f"minv_{nm}", tag="lnminv")
                for ci, (s_ap, q_ap) in enumerate(zip(chunks2(src), chunks2(sq))):
                    px = psS.tile([2, 7 * RW], F32, name=f"px_{nm}{ci}", tag="lnst")
                    pq = psS.tile([2, 7 * RW], F32, name=f"pq_{nm}{ci}", tag="lnst")
                    nc.tensor.matmul(px[:], T["selsum"][:], s_ap)
                    nc.tensor.matmul(pq[:], T["selsum"][:], q_ap)
                    ivc = inv[:, 7 * ci:7 * ci + 7, :]
                    mvc = minv[:, 7 * ci:7 * ci + 7, :]
                    smt = pool.tile([2, 7, RW], F32, name=f"sm_{nm}{ci}", tag="lnsm")
                    nc.vector.tensor_scalar(smt[:], px[:], 1.0 / 64, None, AL.mult)
                    nc.vector.tensor_tensor(ivc, smt[:], smt[:], AL.mult)
                    nc.vector.scalar_tensor_tensor(ivc, pq[:], 1.0 / 64, ivc, AL.mult, AL.subtract)
                    nc.scalar.activation(ivc, ivc, AF.Ln, bias=T["cstc"][0:2, 0:1])
                    nc.scalar.activation(ivc, ivc, AF.Exp, scale=-0.5)
                    nc.vector.tensor_tensor(mvc, smt[:], ivc, AL.mult)
                dst = pool.tile([128, 14, RW], F32, name=f"ln_{nm}")
                for ci in range(2):
                    rs = slice(7 * ci, 7 * ci + 7)
                    sgb = psS.tile([128, 7 * RW], F32, name=f"sgb_{nm}{ci}", tag="lnbc")
                    msgb = psS.tile([128, 7 * RW], F32, name=f"msgb_{nm}{ci}", tag="lnbc")
                    nc.tensor.matmul(sgb[:], T[selg_key][:], inv[:, rs, :])
                    nc.tensor.matmul(msgb[:], T[selg_key][:], minv[:, rs, :])
                    nc.vector.tensor_tensor(dst[:, rs, :], src[:, rs, :],
                                            sgb[:].rearrange("p (a b) -> p a b", a=7), AL.mult)
                    nc.vector.scalar_tensor_tensor(dst[:, rs, :], dst[:, rs, :], T[b_key][:],
                                                   msgb[:].rearrange("p (a b) -> p a b", a=7),
                                                   AL.add, AL.subtract)
                return dst

            def allgather(rnd, cols_src_ap, ncols):
                ci = pool.tile([128, 16], F32, name=f"cci_{rnd}", tag="cci")
                nc.vector.memset(ci[:], 0.0)
                nc.vector.tensor_copy(ci[:, 0:ncols], cols_src_ap)
                nc.sync.dma_start(scc_in[rnd][:], ci[:])
                nc.gpsimd.collective_compute("AllGather", AL.bypass, replica_groups=GRP_ALL,
                                             ins=[scc_in[rnd][:]], outs=[scc_out[rnd][:]])
                gat = pool.tile([128, 16, NCORE], F32, name=f"gat_{rnd}", tag="gat")
                src = scc_out[rnd][:].rearrange("(n p) c -> p c n", p=128)
                nc.sync.dma_start(gat[:, 0:16, :], src)
                return gat

            z1 = ln_ch(ym, "sg_ssm", "sb_ssm", "ssm")
            z2 = ln_ch(z1, "sg_ln2", "sb_ln2", "ln2a")
            pr = pool.tile([128, 14, RW], F32)
            for ci, z_ap in enumerate(chunks2(z2)):
                pp = psT.tile([128, 7 * RW], F32, name=f"pp{ci}", tag="ps1")
                nc.tensor.matmul(pp[:], projT[:], z_ap)
                nc.scalar.activation(pr[:, 7 * ci:7 * ci + 7, :],
                                     pp[:].rearrange("p (a b) -> p a b", a=7),
                                     AF.Relu, bias=T["projb"][:])
            prow = pr[:, OWN[0], OWN[1]]
            sqs = pool.tile([128, 8, 64], F32, name="sqs", tag="sqscratch")
            part0 = pool.tile([128, 3], F32)
            nc.vector.tensor_reduce(part0[:, 0:1], prow, mybir.AxisListType.XY, AL.add)
            nc.scalar.activation(sqs[:], prow, AF.Square, accum_out=part0[:, 1:2])
            nc.vector.tensor_copy(part0[:, 2:3], part0[:, 0:1])
            gat0 = allgather(0, part0[:], 3)
            red0 = pool.tile([128, 4], F32)
            nc.vector.tensor_reduce(red0[:, 0:2], gat0[:, 0:2, :], mybir.AxisListType.X, AL.add)
            pm = pool.tile([128, 16, NCORE], F32, name="pm", tag="pmx")
            nc.vector.tensor_tensor(pm[:, 2:3, :], gat0[:, 2:3, :],
                                    T["bmask"][:].unsqueeze(1), AL.mult)
            nc.vector.tensor_reduce(red0[:, 2:3], pm[:, 2:3, :], mybir.AxisListType.X, AL.add)
            stat0 = psS.tile([128, 4], F32, name="stat0", tag="lnst")
            nc.tensor.matmul(stat0[:, 0:3], T["selhp"][:], red0[:, 0:3])
            mS = pool.tile([128, 6], F32)
            nc.vector.tensor_scalar(mS[:, 0:1], stat0[:, 0:1], 1.0 / NPIX, None, AL.mult)
            nc.vector.tensor_tensor(mS[:, 1:2], mS[:, 0:1], mS[:, 0:1], AL.mult)
            nc.vector.scalar_tensor_tensor(mS[:, 1:2], stat0[:, 1:2], 1.0 / NPIX, mS[:, 1:2],
                                           AL.mult, AL.subtract)
            nc.scalar.activation(mS[:, 1:2], mS[:, 1:2], AF.Ln, bias=eps_ap)
            nc.scalar.activation(mS[:, 1:2], mS[:, 1:2], AF.Exp, scale=-0.5)
            nc.vector.tensor_tensor(mS[:, 1:2], mS[:, 1:2], T["bnp_g"][:], AL.mult)
            nc.vector.tensor_tensor(mS[:, 2:3], mS[:, 0:1], mS[:, 1:2], AL.mult)
            nc.vector.scalar_tensor_tensor(mS[:, 2:3], T["bnp_b"][:], 1.0, mS[:, 2:3],
                                           AL.mult, AL.subtract)
            nc.vector.tensor_scalar(mS[:, 3:4], stat0[:, 2:3], 1.0 / NPB, None, AL.mult)
            nc.vector.tensor_tensor(mS[:, 3:4], mS[:, 3:4], mS[:, 1:2], AL.mult)
            nc.vector.tensor_tensor(mS[:, 3:4], mS[:, 3:4], mS[:, 2:3], AL.add)
            ecp = psS.tile([128, 1], F32, name="ecp", tag="lnst")
            nc.tensor.matmul(ecp[:], T["ecaT"][:], mS[:, 3:4])
            sg = pool.tile([128, 2], F32)
            nc.scalar.activation(sg[:, 0:1], ecp[:], AF.Exp, scale=-1.0)
            nc.vector.tensor_scalar(sg[:, 0:1], sg[:, 0:1], 1.0, None, AL.add)
            nc.vector.reciprocal(sg[:, 1:2], sg[:, 0:1])
            x1 = pool.tile([128, 14, RW], F32)
            nc.vector.tensor_scalar(x1[:], pr[:], mS[:, 1:2], mS[:, 2:3], AL.mult, AL.add)
            nc.vector.scalar_tensor_tensor(x1[:], x1[:], sg[:, 1:2], xt[:], AL.mult, AL.add)
            if dbg:
                nc.sync.dma_start(x1_dbg[:], x1[:])

            m2 = ln_ch(x1, "sg_ln2", "sb_ln2", "ln2b")
            hm = []
            for t_i in range(4):
                hmt = pool.tile([128, 14, RW], F32, name=f"hm{t_i}")
                for ci, m_ap in enumerate(chunks2(m2)):
                    fp = psT.tile([128, 7 * RW], F32, name=f"fp{t_i}{ci}", tag="ps1")
                    nc.tensor.matmul(fp[:], fc1T[:].rearrange("p a b -> p (a b)")
                                     [:, 128 * t_i:128 * t_i + 128], m_ap)
                    nc.scalar.activation(hmt[:, 7 * ci:7 * ci + 7, :],
                                         fp[:].rearrange("p (a b) -> p a b", a=7), AF.Relu)
                hm.append(hmt)
            part1 = pool.tile([128, 8], F32)
            for t_i in range(4):
                nc.vector.tensor_reduce(part1[:, 2 * t_i:2 * t_i + 1], hm[t_i][:, OWN[0], OWN[1]],
                                        mybir.AxisListType.XY, AL.add)
                nc.scalar.activation(sqs[:], hm[t_i][:, OWN[0], OWN[1]], AF.Square,
                                     accum_out=part1[:, 2 * t_i + 1:2 * t_i + 2])
            gat1 = allgather(1, part1[:], 8)
            red1 = pool.tile([128, 8], F32)
            nc.vector.tensor_reduce(red1[:], gat1[:, 0:8, :], mybir.AxisListType.X, AL.add)
            stat1 = psS.tile([128, 8], F32, name="stat1", tag="lnst")
            nc.tensor.matmul(stat1[:], T["selhp"][:], red1[:])
            S1 = pool.tile([128, 4], F32)
            T1 = pool.tile([128, 4], F32)
            for t_i in range(4):
                a, bcol = stat1[:, 2 * t_i:2 * t_i + 1], stat1[:, 2 * t_i + 1:2 * t_i + 2]
                mcol = pool.tile([128, 2], F32, name=f"mcol{t_i}", tag="mcol")
                nc.vector.tensor_scalar(mcol[:, 0:1], a, 1.0 / NPIX, None, AL.mult)
                nc.vector.tensor_tensor(mcol[:, 1:2], mcol[:, 0:1], mcol[:, 0:1], AL.mult)
                nc.vector.scalar_tensor_tensor(mcol[:, 1:2], bcol, 1.0 / NPIX, mcol[:, 1:2],
                                               AL.mult, AL.subtract)
                nc.scalar.activation(mcol[:, 1:2], mcol[:, 1:2], AF.Ln, bias=eps_ap)
                nc.scalar.activation(mcol[:, 1:2], mcol[:, 1:2], AF.Exp, scale=-0.5)
                nc.vector.tensor_tensor(S1[:, t_i:t_i + 1], mcol[:, 1:2],
                                        T["bn1_g"][:, t_i:t_i + 1], AL.mult)
                nc.vector.tensor_tensor(mcol[:, 0:1], mcol[:, 0:1], S1[:, t_i:t_i + 1], AL.mult)
                nc.vector.scalar_tensor_tensor(T1[:, t_i:t_i + 1], T["bn1_b"][:, t_i:t_i + 1],
                                               1.0, mcol[:, 0:1], AL.mult, AL.subtract)
            for t_i in range(4):
                nc.vector.tensor_scalar(hm[t_i][:], hm[t_i][:], S1[:, t_i:t_i + 1],
                                        T1[:, t_i:t_i + 1], AL.mult, AL.add)
                nc.vector.tensor_tensor(hm[t_i][:], hm[t_i][:], cmask[:], AL.mult)

            KS = [1, 3, 5, 7]
            part2 = pool.tile([128, 8], F32)
            r2 = []
            for t_i, ks in enumerate(KS):
                pad = ks // 2
                taps = [(dy, dx) for dy in range(-pad, pad + 1) for dx in range(-pad, pad + 1)]
                acc0 = pool.tile([128, 8, 64], F32, name=f"dacc0_{t_i}", tag="dacc0")
                acc1 = pool.tile([128, 8, 64], F32, name=f"dacc1_{t_i}", tag="dacc1")
                accs = [acc0, acc1]
                cur = 0
                for ti2, (dy, dx) in enumerate(taps):
                    src = hm[t_i][:, 3 + dy:11 + dy, 3 + dx:67 + dx]
                    wap = dw_w[:, t_i, ti2:ti2 + 1]
                    if ti2 == 0:
                        nc.vector.scalar_tensor_tensor(accs[0][:], src, wap,
                                                       hm[t_i][:, OWN[0], OWN[1]], AL.mult, AL.add)
                    else:
                        nc.vector.scalar_tensor_tensor(accs[1 - cur][:], src, wap, accs[cur][:],
                                                       AL.mult, AL.add)
                        cur = 1 - cur
                r2t = pool.tile([128, 8, 64], F32, name=f"r2_{t_i}")
                nc.scalar.activation(r2t[:], accs[cur][:], AF.Relu,
                                     bias=T["dwb"][:, t_i:t_i + 1],
                                     accum_out=part2[:, 2 * t_i:2 * t_i + 1])
                nc.scalar.activation(sqs[:], r2t[:], AF.Square,
                                     accum_out=part2[:, 2 * t_i + 1:2 * t_i + 2])
                r2.append(r2t)
            gat2 = allgather(2, part2[:], 8)
            red2 = pool.tile([128, 8], F32)
            nc.vector.tensor_reduce(red2[:], gat2[:, 0:8, :], mybir.AxisListType.X, AL.add)
            stat2 = psS.tile([128, 8], F32, name="stat2", tag="lnst")
            nc.tensor.matmul(stat2[:], T["selhp"][:], red2[:])
            S2 = pool.tile([128, 4], F32)
            T2 = pool.tile([128, 4], F32)
            for t_i in range(4):
                a, bcol = stat2[:, 2 * t_i:2 * t_i + 1], stat2[:, 2 * t_i + 1:2 * t_i + 2]
                mcol = pool.tile([128, 2], F32, name=f"m2col{t_i}", tag="mcol")
                nc.vector.tensor_scalar(mcol[:, 0:1], a, 1.0 / NPIX, None, AL.mult)
                nc.vector.tensor_tensor(mcol[:, 1:2], mcol[:, 0:1], mcol[:, 0:1], AL.mult)
                nc.vector.scalar_tensor_tensor(mcol[:, 1:2], bcol, 1.0 / NPIX, mcol[:, 1:2],
                                               AL.mult, AL.subtract)
                nc.scalar.activation(mcol[:, 1:2], mcol[:, 1:2], AF.Ln, bias=eps_ap)
                nc.scalar.activation(mcol[:, 1:2], mcol[:, 1:2], AF.Exp, scale=-0.5)
                nc.vector.tensor_tensor(S2[:, t_i:t_i + 1], mcol[:, 1:2],
                                        T["bn2_g"][:, t_i:t_i + 1], AL.mult)
                nc.vector.tensor_tensor(mcol[:, 0:1], mcol[:, 0:1], S2[:, t_i:t_i + 1], AL.mult)
                nc.vector.scalar_tensor_tensor(T2[:, t_i:t_i + 1], T["bn2_b"][:, t_i:t_i + 1],
                                               1.0, mcol[:, 0:1], AL.mult, AL.subtract)
            fo = psT.tile([128, 8, 64], F32, name="fo", tag="dwps")
            for t_i in range(4):
                zt = pool.tile([128, 8, 64], F32, name=f"zt{t_i}", tag="zt", bufs=2)
                nc.vector.tensor_scalar(zt[:], r2[t_i][:], S2[:, t_i:t_i + 1],
                                        T2[:, t_i:t_i + 1], AL.mult, AL.add)
                nc.tensor.matmul(fo[:], fc2T[:, t_i, :], zt[:],
                                 start=(t_i == 0), stop=(t_i == 3), skip_group_check=True)
            fo_sb = pool.tile([128, 8, 64], F32)
            part3 = pool.tile([128, 8], F32)
            nc.scalar.activation(fo_sb[:], fo[:], AF.Copy, accum_out=part3[:, 0:1])
            nc.scalar.activation(sqs[:], fo_sb[:], AF.Square, accum_out=part3[:, 1:2])
            gat3 = allgather(3, part3[:, 0:2], 2)
            red3 = pool.tile([128, 2], F32)
            nc.vector.tensor_reduce(red3[:], gat3[:, 0:2, :], mybir.AxisListType.X, AL.add)
            stat3 = psS.tile([128, 2], F32, name="stat3", tag="lnst")
            nc.tensor.matmul(stat3[:], T["selhp"][:], red3[:])
            mS3 = pool.tile([128, 3], F32)
            nc.vector.tensor_scalar(mS3[:, 0:1], stat3[:, 0:1], 1.0 / NPIX, None, AL.mult)
            nc.vector.tensor_tensor(mS3[:, 1:2], mS3[:, 0:1], mS3[:, 0:1], AL.mult)
            nc.vector.scalar_tensor_tensor(mS3[:, 1:2], stat3[:, 1:2], 1.0 / NPIX, mS3[:, 1:2],
                                           AL.mult, AL.subtract)
            nc.scalar.activation(mS3[:, 1:2], mS3[:, 1:2], AF.Ln, bias=eps_ap)
            nc.scalar.activation(mS3[:, 1:2], mS3[:, 1:2], AF.Exp, scale=-0.5)
            nc.vector.tensor_tensor(mS3[:, 1:2], mS3[:, 1:2], T["bn3_g"][:], AL.mult)
            nc.vector.tensor_tensor(mS3[:, 2:3], mS3[:, 0:1], mS3[:, 1:2], AL.mult)
            nc.vector.scalar_tensor_tensor(mS3[:, 2:3], T["bn3_b"][:], 1.0, mS3[:, 2:3],
                                           AL.mult, AL.subtract)
            fin = pool.tile([128, 8, 64], F32)
            nc.vector.tensor_scalar(fin[:], fo_sb[:], mS3[:, 1:2], mS3[:, 2:3], AL.mult, AL.add)
            nc.vector.tensor_tensor(fin[:], fin[:], x1[:, OWN[0], OWN[1]], AL.add)
            fbf = pool.tile([128, 8, 64], BF16)
            nc.scalar.copy(fbf[:], fin[:])
            for h in (0, 1):
                nc.sync.dma_start(og_in[:, 8 * 64 * h:8 * 64 * h + 512],
                                  fbf[64 * h:64 * h + 64].rearrange("p a b -> p (a b)"))
            nc.gpsimd.collective_compute("AllGather", AL.bypass, replica_groups=GRP_ALL,
                                         ins=[og_in[:]], outs=[og_cc[:]])
            ob = pool.tile([128, 4, 1024], BF16)
            for blk in range(4):
                nc.sync.dma_start(ob[:, blk, :], og_cc[128 * blk:128 * blk + 128, :])
            for blk in range(4):
                nc.sync.dma_start(outg[128 * blk:128 * blk + 128, :], ob[:, blk, :])
    _fix_multiwaits(nc)
    return nc

# ============================================================ host-side prep

def _f(a):
    return np.ascontiguousarray(a, dtype=np.float32)


def prep_static(W_):
    """All inputs except xs, per core. W_ = dict of original kernel inputs."""
    ln1_g, ln1_b = _f(W_["ln1_g"]), _f(W_["ln1_b"])
    con1_w, con1_b = _f(W_["con1_w"]), _f(W_["con1_b"])
    xproj_w, dtproj_w = _f(W_["xproj_w"]), _f(W_["dtproj_w"])
    dtproj_b, A_log, Ds = _f(W_["dtproj_b"]), _f(W_["A_log"]), _f(W_["Ds"])
    A = -np.exp(A_log)

    cc = np.arange(64)
    # L1 shared
    selsum = np.zeros((128, 2), np.float32)
    selsum[:64, 0] = 1.0; selsum[64:, 1] = 1.0
    selg = np.zeros((2, 128), np.float32)
    selg[0, :64] = ln1_g; selg[1, 64:] = ln1_g
    b2 = np.tile(ln1_b, 2)[:, None].astype(np.float32)
    w9 = np.tile(con1_w[:, 0].reshape(64, 9), (2, 1)).astype(np.float32)
    cbv = np.tile(con1_b, 2)[:, None].astype(np.float32)
    cst2 = np.zeros((2, 2), np.float32); cst2[:, 0] = EPS

    # L3 shared
    sg_ssm = np.zeros((2, 128), np.float32)
    sg_ssm[0, :64] = W_["ssm_ln_g"]; sg_ssm[1, 64:] = W_["ssm_ln_g"]
    sb_ssm = np.tile(_f(W_["ssm_ln_b"]), 2)[:, None].astype(np.float32)
    sg_ln2 = np.zeros((2, 128), np.float32)
    sg_ln2[0, :64] = W_["ln2_g"]; sg_ln2[1, 64:] = W_["ln2_g"]
    sb_ln2 = np.tile(_f(W_["ln2_b"]), 2)[:, None].astype(np.float32)
    selhp = np.tile(np.eye(64, dtype=np.float32), (2, 2))
    ew = _f(W_["eca_w"])
    band = (ew[1] * np.eye(64) + ew[2] * np.eye(64, k=-1) + ew[0] * np.eye(64, k=1))
    ecaT = np.zeros((128, 128), np.float32)
    ecaT[0:64, 0:64] = band; ecaT[0:64, 64:128] = band
    projc = _f(W_["proj_w"][:, :, 0, 0].T)
    projb = np.tile(_f(W_["proj_b"]), 2)[:, None].astype(np.float32)
    bnp_g = np.tile(_f(W_["proj_bn_g"]), 2)[:, None].astype(np.float32)
    bnp_b = np.tile(_f(W_["proj_bn_b"]), 2)[:, None].astype(np.float32)
    fc1c = _f(W_["fc1_w"][:, :, 0, 0].T)           # (64, 256): [c, 64t+c2]
    fc2c = _f(np.transpose(W_["fc2_w"][:, :, 0, 0].T.reshape(4, 64, 64), (1, 0, 2)))
    bn1_g = _f(W_["bn1_g"]).reshape(4, 64).T
    bn1_g = np.tile(bn1_g, (2, 1)).astype(np.float32)
    bn1_b = np.tile(_f(W_["bn1_b"]).reshape(4, 64).T, (2, 1)).astype(np.float32)
    bn2_g = np.tile(_f(W_["bn2_g"]).reshape(4, 64).T, (2, 1)).astype(np.float32)
    bn2_b = np.tile(_f(W_["bn2_b"]).reshape(4, 64).T, (2, 1)).astype(np.float32)
    bn3_g = np.tile(_f(W_["bn3_g"]), 2)[:, None].astype(np.float32)
    bn3_b = np.tile(_f(W_["bn3_b"]), 2)[:, None].astype(np.float32)
    dwk = [_f(W_["dw_w1"]), _f(W_["dw_w3"]), _f(W_["dw_w5"]), _f(W_["dw_w7"])]
    dwbs = [_f(W_["dw_b1"]), _f(W_["dw_b3"]), _f(W_["dw_b5"]), _f(W_["dw_b7"])]
    dwc = np.zeros((64, 4, 49), np.float32)
    for t in range(4):
        ks = 2 * t + 1
        dwc[:, t, :ks * ks] = dwk[t][:, 0].reshape(64, ks * ks)
    dwb = np.tile(np.stack(dwbs, 1), (2, 1)).astype(np.float32)
    cstc = np.zeros((128, 2), np.float32); cstc[:, 0] = EPS
    i32 = np.eye(D2, dtype=np.float32)

    shared = dict(selsum=selsum, selg=selg, b2=b2, w9=w9, cb=cbv, cst2=cst2,
                  sg_ssm=sg_ssm, sb_ssm=sb_ssm, sg_ln2=sg_ln2, sb_ln2=sb_ln2,
                  selhp=selhp, ecaT=ecaT, projc=projc, projb=projb,
                  bnp_g=bnp_g, bnp_b=bnp_b, fc1c=fc1c, fc2c=fc2c,
                  bn1_g=bn1_g, bn1_b=bn1_b, bn2_g=bn2_g, bn2_b=bn2_b,
                  bn3_g=bn3_g, bn3_b=bn3_b, dwc=dwc, dwb=dwb, cstc=cstc, i32=i32)

    # per-direction L2 weights
    l2k = []
    for k in range(4):
        Wd = dtproj_w[k] @ xproj_w[k, :R]          # (32, 32): [d, e]
        WdT = _f(Wd.T)
        bgen = _f(xproj_w[k, R:R + N].T)           # (32e, 16n) n=4g+j
        cgen = _f(xproj_w[k, R + N:].T)
        bias4 = np.tile(dtproj_b[k], 4)[:, None].astype(np.float32)
        A4 = _f(np.transpose(A[k].reshape(D2, 4, 4), (2, 0, 1)).reshape(128, 4))
        dsc = _f(Ds[k])[:, None]
        kmv = np.zeros((D2, 4), np.float32)
        kmv[:, 0] = 1.0 if k < 2 else 0.0
        kmv[:, 1] = 1.0 if k >= 2 else 0.0
        kmv[:, 2] = 1.0 if k % 2 == 0 else 0.0
        kmv[:, 3] = 1.0 if k % 2 == 1 else 0.0
        l2k.append(dict(WdT=WdT, bgen=bgen, cgen=cgen, bias4=bias4, A4=A4,
                        dsc=dsc, kmask=kmv))

    maps = []
    pp = np.arange(128)
    hh, ch = pp // 64, pp % 64
    for i in range(NCORE):
        b, q = i // 4, i % 4
        # l1mask: valid global row for conv window stored row r (12 per half)
        r = np.arange(12)
        g1m = 16 * q - 2 + 8 * hh[:, None] + r[None, :]
        l1mask = ((g1m >= 0) & (g1m < H)).astype(np.float32)
        s = np.arange(14)
        g3m = 16 * q - 3 + 8 * hh[:, None] + s[None, :]
        rowmask14 = ((g3m >= 0) & (g3m < H)).astype(np.float32)
        bmask = np.zeros((128, 8), np.float32)
        bmask[:, 4 * b:4 * b + 4] = 1.0
        qmask = np.zeros((128, 4), np.float32)
        qmask[:, q] = 1.0
        maps.append(dict(shared, **l2k[i % 4], l1mask=l1mask,
                         rowmask14=rowmask14, bmask=bmask, qmask=qmask))
    return maps


def prep_dyn(x):
    """Packed x slices for all cores (22 rows with halo, zero row-padded).
    Uploaded only to core 0; an on-device ReduceScatter hands out slices."""
    dt = mybir.dt.np(BF16) if XS_BF16 else np.float32
    xpack = np.zeros((NCORE * C, 22, W), dt)
    for i in range(NCORE):
        b, q = i // 4, i % 4
        lo = 16 * q - 3
        slo, shi = max(lo, 0), min(lo + 22, H)
        xpack[64 * i:64 * i + 64, slo - lo:shi - lo, :] = \
            x[b, :, slo:shi, :].astype(dt)
    return xpack

# ============================================================ cached runner

class _Runner:
    """Mirror of bass2jax.run_bass_via_pjrt with a persistent jitted callable
    and device-resident static (weight) inputs across calls."""

    def __init__(self, nc, n_cores, static_names):
        import jax
        from jax.sharding import Mesh, PartitionSpec, NamedSharding
        from jax.experimental.shard_map import shard_map
        from concourse import bass2jax
        bass2jax.install_neuronx_cc_hook()
        self.nc = nc
        self.n_cores = n_cores
        partition_name = nc.partition_id_tensor.name if nc.partition_id_tensor else None
        in_names, out_names, out_avals, zero_outs = [], [], [], []
        for alloc in nc.m.functions[0].allocations:
            if not isinstance(alloc, mybir.MemoryLocationSet):
                continue
            name = alloc.memorylocations[0].name
            if alloc.kind == "ExternalInput":
                if name != partition_name:
                    in_names.append(name)
            elif alloc.kind == "ExternalOutput":
                shape = tuple(alloc.tensor_shape)
                dtype = mybir.dt.np(alloc.dtype)
                out_names.append(name)
                out_avals.append(jax.core.ShapedArray(shape, dtype))
                zero_outs.append(np.zeros(shape, dtype))
        self.in_names = list(in_names)
        self.out_names = out_names
        self.out_avals = out_avals
        self.zero_outs = zero_outs
        n_params = len(in_names)
        all_in = in_names + out_names + ([partition_name] if partition_name else [])
        donate = tuple(range(n_params, n_params + len(out_names)))
        self.n_params = n_params
        self.static_idx = [j for j, nm in enumerate(in_names) if nm in static_names]
        self.static_names = static_names

        def _body(*args):
            operands = list(args)
            if partition_name is not None:
                operands.append(bass2jax.partition_id_tensor())
            outs = bass2jax._bass_exec_p.bind(
                *operands,
                out_avals=tuple(out_avals),
                in_names=tuple(all_in),
                out_names=tuple(out_names),
                lowering_input_output_aliases=(),
                sim_require_finite=True,
                sim_require_nnan=True,
                nc=nc,
            )
            return tuple(outs)

        devices = jax.devices()[:n_cores]
        assert len(devices) == n_cores
        self.mesh = Mesh(np.asarray(devices), ("core",))
        in_specs = (PartitionSpec("core"),) * (n_params + len(out_names))
        out_specs = (PartitionSpec("core"),) * len(out_names)
        self.sharding = NamedSharding(self.mesh, PartitionSpec("core"))
        self.jitted = jax.jit(
            shard_map(_body, mesh=self.mesh, in_specs=in_specs,
                      out_specs=out_specs, check_rep=False),
            donate_argnums=donate, keep_unused=True)
        self._static_cache = {}   # name -> device_array (+ "key")
        self._jax = jax
        self._devices = list(devices)
        # donated output buffers created on device (no host->device upload)
        import jax.numpy as jnp
        zshapes = [((n_cores * z.shape[0],) + z.shape[1:], z.dtype)
                   for z in zero_outs]
        self._zeros_fn = jax.jit(
            lambda: tuple(jnp.zeros(s, d) for s, d in zshapes),
            out_shardings=tuple(self.sharding for _ in zshapes))
        # persistent device-resident zero shards for the scatter input on
        # cores 1..n-1 (never donated, so created once)
        self._xzeros = None

    def _upload_scatter(self, shard0):
        """Assemble the global 'xall' array: real data on device 0, cached
        zero shards on the rest. One host->device transfer total."""
        jax = self._jax
        import jax.numpy as jnp
        from jax.sharding import SingleDeviceSharding
        if self._xzeros is None:
            zs = []
            for dev in self._devices[1:]:
                fn = jax.jit(lambda: jnp.zeros(shard0.shape, shard0.dtype),
                             out_shardings=SingleDeviceSharding(dev))
                zs.append(fn())
            self._xzeros = zs
        d0 = jax.device_put(shard0, self._devices[0])
        gshape = (self.n_cores * shard0.shape[0],) + shard0.shape[1:]
        return jax.make_array_from_single_device_arrays(
            gshape, self.sharding, [d0] + self._xzeros)

    def _concat(self, in_maps, name):
        return np.concatenate([np.asarray(m[name]) for m in in_maps], axis=0)

    def __call__(self, static_maps, xall_shard0, static_key=None):
        """static_maps: per-core dicts of static inputs. xall_shard0: the
        packed dynamic input, uploaded to device 0 only. Outputs in
        self.gather_outs are replicated on device; only shard 0 is fetched."""
        # kick off the async device work first: donated-zeros creation and
        # the single-shard x upload both overlap the static-arg assembly
        # and each other, keeping only one RTT on the critical path.
        zz = self._zeros_fn()
        xall_dev = self._upload_scatter(xall_shard0)
        args = []
        use_cache = static_key is not None and static_key == self._static_cache.get("key")
        new_static = {}
        for nm in self.in_names:
            if nm == "xall":
                args.append(xall_dev)
            elif use_cache:
                args.append(self._static_cache[nm])
            else:
                dev = self._jax.device_put(self._concat(static_maps, nm), self.sharding)
                new_static[nm] = dev
                args.append(dev)
        if not use_cache:
            new_static["key"] = static_key
            self._static_cache = new_static
        out_arrs = self.jitted(*args, *zz)
        res = {}
        for i, nm in enumerate(self.out_names):
            if nm == "outg":
                shards = sorted(out_arrs[i].addressable_shards, key=lambda s: s.device.id)
                res[nm] = np.asarray(shards[0].data)
            else:   # debug outputs: fetch every core's shard
                res[nm] = np.asarray(out_arrs[i]).reshape(
                    self.n_cores, *self.out_avals[i].shape)
        return res

# ============================================================ kernel()

_STATE = {}
TRACE = False
LAST_EXEC_NS = []


def _get_runner():
    if "runner" not in _STATE:
        nc = build_fused(dbg=DBG)
        static = {n for n in (
            "l1mask", "selsum", "selg", "b2", "w9", "cb", "cst2",
            "WdT", "bgen", "cgen", "bias4", "A4", "dsc", "i32", "kmask",
            "rowmask14", "bmask", "qmask", "selhp", "ecaT",
            "sg_ssm", "sb_ssm", "sg_ln2", "sb_ln2", "projc", "projb",
            "bnp_g", "bnp_b", "fc1c", "bn1_g", "bn1_b", "dwc", "dwb",
            "bn2_g", "bn2_b", "fc2c", "bn3_g", "bn3_b", "cstc")}
        _STATE["runner"] = _Runner(nc, NCORE, static)
        _STATE["nc"] = nc
    return _STATE["runner"]


def _weights_key(W_):
    """Fingerprint of everything except x (the static/weight inputs)."""
    import zlib
    h = 0
    for k in sorted(W_):
        if k == "x":
            continue
        a = np.ascontiguousarray(W_[k])
        h = zlib.crc32(a.tobytes(), zlib.crc32(k.encode(), h))
    return h


def kernel(**inputs):
    import time as _t
    W_ = {k: np.asarray(v) for k, v in inputs.items()}
    x = np.asarray(W_["x"], np.float32)
    runner = _get_runner()

    wkey = _weights_key(W_)
    if _STATE.get("wkey") != wkey:
        _STATE["static_maps"] = prep_static(W_)
        _STATE["wkey"] = wkey
    sm = _STATE["static_maps"]
    xpack = prep_dyn(x)

    LAST_EXEC_NS.clear()
    t0 = _t.perf_counter()
    try:
        results = runner(sm, xpack, static_key=wkey)
        outg = results["outg"]
    except Exception:
        from concourse.bass_utils import run_bass_kernel_spmd
        maps = [dict(sm[i], xall=(xpack if i == 0 else np.zeros_like(xpack)))
                for i in range(NCORE)]
        r = run_bass_kernel_spmd(_STATE["nc"], maps, list(range(NCORE))).results
        outg = r[0]["outg"]
        results = {"outg": outg, "percore": r}
    if TRACE:
        LAST_EXEC_NS.append(int((_t.perf_counter() - t0) * 1e9))

    o = np.zeros((B, C, H, W), np.float32)
    og = np.asarray(outg).astype(np.float32)
    for i in range(NCORE):
        b, q = i // 4, i % 4
        o[b, :, 16 * q:16 * q + 16, :] = og[64 * i:64 * i + 64].reshape(C, 16, W)
    if DBG and isinstance(results, dict):
        _STATE["dbg"] = [{nm: v[i] for nm, v in results.items()
                          if nm not in ("outg", "percore")} for i in range(NCORE)]
    return o
